# revision 5
# baseline (speedup 1.0000x reference)
"""Chamfer loss (nn_ChamferLoss) on 8 Trainium2 NeuronCores.

Strategy
--------
d2[i,j] = |a_i|^2 + |b_j|^2 - 2 a_i.b_j is computed on the TensorEngine as a
single K=24 bf16 matmul per tile: each fp32 operand is split into three bf16
limbs (h/m/l, 8 mantissa bits each -> 24 bits, fp32-exact), and the six
significant limb-product pairs plus the two squared-norm rows (3 limbs each,
paired with ones) are stacked along the contraction dim.  bf16 streams at
1 col/cycle on the PE (fp32 would be 4x slower) and PSUM accumulates in fp32,
so the distance tile is fp32-accurate at full PE rate.

Both directions of the Chamfer min are computed as *row*-min passes:
  pass 0: rows = this core's 1024 target points, cols = all 8192 output pts
  pass 1: rows = this core's 1024 output points, cols = all 8192 target pts
so only free-axis reductions are needed (no cross-core or cross-partition
min).  The row-min uses DVE tensor_tensor_reduce, which consumes TWO psum
tiles per cycle (elementwise min via op0 + running free-axis min via op1),
halving the DVE cost versus plain tensor_reduce.

Each core returns sqrt(relu(rowmin)) per point plus its two partial sums;
the host adds 16 numbers and applies the scale.
"""

import sys

sys.path.insert(0, "/opt/trn_rl_repo")

import numpy as np
import ml_dtypes

N = 8192           # points per cloud
D = 3
NCORES = 8
NPC = N // NCORES  # 1024 rows per core per pass
P = 128            # partitions
BLKS = NPC // P    # 8 row blocks per pass
K = 24             # contraction rows (6 limb pairs * 3 coords + 2 norms * 3 limbs)
CH = 512           # matmul free dim (one PSUM bank of fp32)
NCH = N // CH      # 16 chunks per row sweep
TPB = 4            # psum tiles per block (each tile = 4 chunks = 2048 cols)

_BUILT = None


def _limbs(x):
    """Split fp32 array into three bf16 limbs whose sum is (near-)exact."""
    h = x.astype(ml_dtypes.bfloat16).astype(np.float32)
    r = x - h
    m = r.astype(ml_dtypes.bfloat16).astype(np.float32)
    l = (r - m).astype(ml_dtypes.bfloat16).astype(np.float32)
    return h, m, l


def _stationary_rows(pts):
    """[24, n] lhsT rows for the 'row' cloud: coords + |p|^2 limbs + ones."""
    ph, pm, pl = _limbs(pts)                       # (n,3) each
    p2 = np.sum(pts.astype(np.float64) ** 2, -1).astype(np.float32)
    p2h, p2m, p2l = _limbs(p2)
    one = np.ones_like(p2)
    return np.stack(
        [ph[:, 0], ph[:, 1], ph[:, 2],
         ph[:, 0], ph[:, 1], ph[:, 2],
         pm[:, 0], pm[:, 1], pm[:, 2],
         ph[:, 0], ph[:, 1], ph[:, 2],
         pl[:, 0], pl[:, 1], pl[:, 2],
         pm[:, 0], pm[:, 1], pm[:, 2],
         p2h, p2m, p2l,
         one, one, one], 0)


def _moving_rows(pts):
    """[24, n] rhs rows for the 'column' cloud, limb-paired with the above."""
    qh, qm, ql = _limbs(pts)
    q2 = np.sum(pts.astype(np.float64) ** 2, -1).astype(np.float32)
    q2h, q2m, q2l = _limbs(q2)
    one = np.ones_like(q2)
    return np.stack(
        [-2 * qh[:, 0], -2 * qh[:, 1], -2 * qh[:, 2],
         -2 * qm[:, 0], -2 * qm[:, 1], -2 * qm[:, 2],
         -2 * qh[:, 0], -2 * qh[:, 1], -2 * qh[:, 2],
         -2 * ql[:, 0], -2 * ql[:, 1], -2 * ql[:, 2],
         -2 * qh[:, 0], -2 * qh[:, 1], -2 * qh[:, 2],
         -2 * qm[:, 0], -2 * qm[:, 1], -2 * qm[:, 2],
         one, one, one,
         q2h, q2m, q2l], 0)


def _build():
    global _BUILT
    if _BUILT is not None:
        return _BUILT

    import concourse.bacc as bacc
    import concourse.mybir as mybir
    import concourse.tile as tile
    from concourse.bass_isa import ReduceOp

    f32 = mybir.dt.float32
    bf16 = mybir.dt.bfloat16
    MIN = mybir.AluOpType.min
    X = mybir.AxisListType.X

    nc = bacc.Bacc(None, target_bir_lowering=False, debug=False)
    wts = nc.declare_dram_parameter("wts", [2, K, NPC], bf16, isOutput=False)
    rhs = nc.declare_dram_parameter("rhs", [2, K, N], bf16, isOutput=False)
    mins_out = nc.declare_dram_parameter("mins", [P, 2 * BLKS], f32, isOutput=True)
    sums_out = nc.declare_dram_parameter("sums", [1, 2], f32, isOutput=True)

    with tile.TileContext(nc) as tc:
        with tc.tile_pool(name="const", bufs=1) as cpool, \
             tc.tile_pool(name="acc", bufs=4) as apool, \
             tc.tile_pool(name="ps", bufs=2, space="PSUM") as pspool:
            w_t = cpool.tile([K, 2 * NPC], bf16, name="w_t")
            r_t = cpool.tile([K, 2 * N], bf16, name="r_t")
            nc.sync.dma_start(out=w_t[:, 0:NPC], in_=wts[0])
            nc.sync.dma_start(out=w_t[:, NPC:2 * NPC], in_=wts[1])
            nc.sync.dma_start(out=r_t[:, 0:N], in_=rhs[0])
            nc.sync.dma_start(out=r_t[:, N:2 * N], in_=rhs[1])

            mins16 = cpool.tile([P, 2 * BLKS], f32, name="mins16")
            sq16 = cpool.tile([P, 2 * BLKS], f32, name="sq16")
            sums2 = cpool.tile([P, 2], f32, name="sums2")

            for ps_idx in range(2):           # pass 0: target rows, 1: output rows
                for b in range(BLKS):
                    acc4 = apool.tile([P, TPB], f32, name="acc4")
                    lhsT = w_t[:, ps_idx * NPC + b * P: ps_idx * NPC + (b + 1) * P]
                    for t in range(TPB):
                        pst = pspool.tile([P, 4 * CH], f32, name="pst")
                        for j in range(4):
                            c = t * 4 + j
                            nc.tensor.matmul(
                                out=pst[:, j * CH:(j + 1) * CH],
                                lhsT=lhsT,
                                rhs=r_t[:, ps_idx * N + c * CH: ps_idx * N + (c + 1) * CH],
                            )
                        nc.vector.tensor_reduce(
                            out=acc4[:, t:t + 1], in_=pst, axis=X, op=MIN)
                    col = ps_idx * BLKS + b
                    nc.vector.tensor_reduce(
                        out=mins16[:, col:col + 1], in_=acc4, axis=X, op=MIN)

            # sqrt(relu(min)) and per-core partial sums
            nc.vector.tensor_scalar_max(out=mins16, in0=mins16, scalar1=0.0)
            nc.scalar.sqrt(sq16, mins16)
            nc.vector.tensor_reduce(
                out=sums2[:, 0:1], in_=sq16[:, 0:BLKS], axis=X, op=mybir.AluOpType.add)
            nc.vector.tensor_reduce(
                out=sums2[:, 1:2], in_=sq16[:, BLKS:2 * BLKS], axis=X, op=mybir.AluOpType.add)
            nc.gpsimd.partition_all_reduce(sums2, sums2, P, ReduceOp.add)
            nc.sync.dma_start(out=mins_out[:, :], in_=sq16[:, :])
            nc.sync.dma_start(out=sums_out[:, :], in_=sums2[0:1, :])

    nc.compile()
    _BUILT = nc
    return nc


def kernel(target, output, cur, substeps):
    from concourse.bass_utils import run_bass_kernel_spmd

    a = np.asarray(target, dtype=np.float32)[0]   # (8192, 3) target cloud
    b = np.asarray(output, dtype=np.float32)[0]   # (8192, 3) output cloud
    cur = int(np.asarray(cur))
    substeps = int(np.asarray(substeps))

    bf = ml_dtypes.bfloat16
    w_pass0 = _stationary_rows(a).astype(bf)      # [24, 8192] rows = targets
    r_pass0 = _moving_rows(b).astype(bf)          # [24, 8192] cols = outputs
    w_pass1 = _stationary_rows(b).astype(bf)      # rows = outputs
    r_pass1 = _moving_rows(a).astype(bf)          # cols = targets

    rhs = np.stack([r_pass0, r_pass1])            # same on every core
    in_maps = []
    for c in range(NCORES):
        sl = slice(c * NPC, (c + 1) * NPC)
        in_maps.append({
            "wts": np.stack([w_pass0[:, sl], w_pass1[:, sl]]),
            "rhs": rhs,
        })

    nc = _build()
    results = run_bass_kernel_spmd(nc, in_maps, list(range(NCORES))).results

    s1 = 0.0
    s2 = 0.0
    for c in range(NCORES):
        s = results[c]["sums"]
        s1 += float(s[0, 0])
        s2 += float(s[0, 1])
    loss = 0.5 * (s1 / N + s2 / N)
    scale = 10.0 / (0.99 ** (cur // substeps))
    return np.float32(loss * scale)


# revision 6
# speedup vs baseline: 1.3985x; 1.3985x over previous
"""Chamfer loss (nn_ChamferLoss) on 8 Trainium2 NeuronCores.

V2: one-matrix formulation. Each core computes its 1024-target-row stripe of
the 8192x8192 squared-distance matrix exactly once as K=24 bf16 matmuls (3-limb
bf16 decomposition of the fp32 operands -> fp32-accurate d2 at full PE rate).

Engine split per [128,2048] PSUM tile:
  PE   : 16 back-to-back matmuls per 128-row block (shared weights)
  ACT  : evacuates PSUM -> SBUF bf16 copies (scalar engine copy)
  DVE  : row-min  = bf16 tensor_tensor min tree over the block's 4 copies
                    (2x packed mode) + one small reduce  -> dist1 row mins
         col-min  = bf16 tensor_tensor min of each copy into a running
                    column accumulator (min over this core's rows)
Host: gathers row mins (dist1) and the 8 cores' [128,8192] column accumulators
(dist2 = min over cores x partitions), sqrt/mean/scale epilogue.
"""

import sys

sys.path.insert(0, "/opt/trn_rl_repo")

import numpy as np
import ml_dtypes

N = 8192           # points per cloud
NCORES = 8
NPC = N // NCORES  # 1024 rows per core
P = 128
BLKS = NPC // P    # 8 blocks
K = 24             # contraction rows
CH = 512           # matmul free dim (one PSUM bank fp32)
TW = 2048          # psum tile width (4 banks)
TPB = N // TW      # 4 psum tiles per block row sweep

_BUILT = None


def _limbs(x):
    h = x.astype(ml_dtypes.bfloat16).astype(np.float32)
    r = x - h
    m = r.astype(ml_dtypes.bfloat16).astype(np.float32)
    l = (r - m).astype(ml_dtypes.bfloat16).astype(np.float32)
    return h, m, l


def _stationary_rows(pts):
    """[24, n] lhsT rows: coord limbs + |p|^2 limbs + ones."""
    ph, pm, pl = _limbs(pts)
    p2 = np.sum(pts.astype(np.float64) ** 2, -1).astype(np.float32)
    p2h, p2m, p2l = _limbs(p2)
    one = np.ones_like(p2)
    return np.stack(
        [ph[:, 0], ph[:, 1], ph[:, 2],
         ph[:, 0], ph[:, 1], ph[:, 2],
         pm[:, 0], pm[:, 1], pm[:, 2],
         ph[:, 0], ph[:, 1], ph[:, 2],
         pl[:, 0], pl[:, 1], pl[:, 2],
         pm[:, 0], pm[:, 1], pm[:, 2],
         p2h, p2m, p2l,
         one, one, one], 0)


def _moving_rows(pts):
    """[24, n] rhs rows, limb-paired with _stationary_rows."""
    qh, qm, ql = _limbs(pts)
    q2 = np.sum(pts.astype(np.float64) ** 2, -1).astype(np.float32)
    q2h, q2m, q2l = _limbs(q2)
    one = np.ones_like(q2)
    return np.stack(
        [-2 * qh[:, 0], -2 * qh[:, 1], -2 * qh[:, 2],
         -2 * qm[:, 0], -2 * qm[:, 1], -2 * qm[:, 2],
         -2 * qh[:, 0], -2 * qh[:, 1], -2 * qh[:, 2],
         -2 * ql[:, 0], -2 * ql[:, 1], -2 * ql[:, 2],
         -2 * qh[:, 0], -2 * qh[:, 1], -2 * qh[:, 2],
         -2 * qm[:, 0], -2 * qm[:, 1], -2 * qm[:, 2],
         one, one, one,
         q2h, q2m, q2l], 0)


def _build():
    global _BUILT
    if _BUILT is not None:
        return _BUILT

    import concourse.bacc as bacc
    import concourse.mybir as mybir
    import concourse.tile as tile

    f32 = mybir.dt.float32
    bf16 = mybir.dt.bfloat16
    MIN = mybir.AluOpType.min
    X = mybir.AxisListType.X

    nc = bacc.Bacc(None, target_bir_lowering=False, debug=False)
    wts = nc.declare_dram_parameter("wts", [K, NPC], bf16, isOutput=False)
    rhs = nc.declare_dram_parameter("rhs", [K, N], bf16, isOutput=False)
    rowout_d = nc.declare_dram_parameter("rowout", [P, BLKS], f32, isOutput=True)
    colout_d = nc.declare_dram_parameter("colout", [P, N], bf16, isOutput=True)

    with tile.TileContext(nc) as tc:
        with tc.tile_pool(name="const", bufs=1) as cpool, \
             tc.tile_pool(name="cp", bufs=6) as cppool, \
             tc.tile_pool(name="ps", bufs=2, space="PSUM") as pspool:
            w_t = cpool.tile([K, NPC], bf16, name="w_t")
            r_t = cpool.tile([K, N], bf16, name="r_t")
            nc.sync.dma_start(out=w_t[:, :], in_=wts[:, :])
            nc.sync.dma_start(out=r_t[:, :], in_=rhs[:, :])

            # column accumulators, one per 2048-wide chunk group
            colacc = [cpool.tile([P, TW], bf16, name=f"colacc{cc}")
                      for cc in range(TPB)]
            s1 = cpool.tile([P, TW], bf16, name="s1")
            s2 = cpool.tile([P, TW], bf16, name="s2")
            rowout = cpool.tile([P, BLKS], f32, name="rowout")

            for b in range(BLKS):
                lhsT = w_t[:, b * P:(b + 1) * P]
                copies = []
                for cc in range(TPB):
                    pst = pspool.tile([P, TW], f32, name="pst")
                    for j in range(TW // CH):
                        c = cc * (TW // CH) + j
                        nc.tensor.matmul(
                            out=pst[:, j * CH:(j + 1) * CH],
                            lhsT=lhsT,
                            rhs=r_t[:, c * CH:(c + 1) * CH],
                        )
                    if b == 0:
                        cp_t = colacc[cc]     # block 0 copies ARE the init
                    else:
                        cp_t = cppool.tile([P, TW], bf16, name="cp", tag="cp")
                    nc.scalar.copy(out=cp_t[:, :], in_=pst[:, :])
                    copies.append(cp_t)

                # row-min tree over the block's 4 copies (bf16 2x TT mode)
                nc.vector.tensor_tensor(out=s1, in0=copies[0], in1=copies[1], op=MIN)
                nc.vector.tensor_tensor(out=s2, in0=copies[2], in1=copies[3], op=MIN)
                nc.vector.tensor_tensor(out=s1, in0=s1, in1=s2, op=MIN)
                nc.vector.tensor_tensor(
                    out=s2[:, 0:TW // 2], in0=s1[:, 0:TW // 2], in1=s1[:, TW // 2:TW], op=MIN)
                nc.vector.tensor_tensor(
                    out=s1[:, 0:TW // 4], in0=s2[:, 0:TW // 4], in1=s2[:, TW // 4:TW // 2], op=MIN)
                nc.vector.tensor_reduce(
                    out=rowout[:, b:b + 1], in_=s1[:, 0:TW // 4], axis=X, op=MIN)

                # column accumulation (skip block 0 — copies initialized it)
                if b > 0:
                    for cc in range(TPB):
                        nc.vector.tensor_tensor(
                            out=colacc[cc], in0=copies[cc], in1=colacc[cc], op=MIN)

            nc.sync.dma_start(out=rowout_d[:, :], in_=rowout[:, :])
            for cc in range(TPB):
                nc.sync.dma_start(
                    out=colout_d[:, cc * TW:(cc + 1) * TW], in_=colacc[cc][:, :])

    nc.compile()
    _BUILT = nc
    return nc


def kernel(target, output, cur, substeps):
    from concourse.bass_utils import run_bass_kernel_spmd

    a = np.asarray(target, dtype=np.float32)[0]   # (8192, 3) target cloud
    b = np.asarray(output, dtype=np.float32)[0]   # (8192, 3) output cloud
    cur = int(np.asarray(cur))
    substeps = int(np.asarray(substeps))

    bf = ml_dtypes.bfloat16
    w_full = _stationary_rows(a).astype(bf)       # [24, 8192] rows = targets
    r_full = _moving_rows(b).astype(bf)           # [24, 8192] cols = outputs

    in_maps = []
    for c in range(NCORES):
        sl = slice(c * NPC, (c + 1) * NPC)
        in_maps.append({"wts": np.ascontiguousarray(w_full[:, sl]),
                        "rhs": r_full})

    nc = _build()
    results = run_bass_kernel_spmd(nc, in_maps, list(range(NCORES))).results

    # dist1: per-target row minima (already min over all output points)
    d1 = np.concatenate(
        [results[c]["rowout"].T.reshape(-1) for c in range(NCORES)])  # (8192,)
    # dist2: fold cores and partitions of the column accumulators
    colmins = np.stack([
        results[c]["colout"].astype(np.float32).min(axis=0)
        for c in range(NCORES)])                                      # (8, 8192)
    d2 = colmins.min(axis=0)

    m1 = np.sqrt(np.maximum(d1, 0.0)).mean()
    m2 = np.sqrt(np.maximum(d2, 0.0)).mean()
    loss = 0.5 * (m1 + m2)
    scale = 10.0 / (0.99 ** (cur // substeps))
    return np.float32(loss * scale)


# revision 7
# speedup vs baseline: 1.6797x; 1.2011x over previous
"""Chamfer loss (nn_ChamferLoss) on 8 Trainium2 NeuronCores.

V2: one-matrix formulation. Each core computes its 1024-target-row stripe of
the 8192x8192 squared-distance matrix exactly once as K=24 bf16 matmuls (3-limb
bf16 decomposition of the fp32 operands -> fp32-accurate d2 at full PE rate).

Engine split per [128,2048] PSUM tile:
  PE   : 16 back-to-back matmuls per 128-row block (shared weights)
  ACT  : evacuates PSUM -> SBUF bf16 copies (scalar engine copy)
  DVE  : row-min  = bf16 tensor_tensor min tree over the block's 4 copies
                    (2x packed mode) + one small reduce  -> dist1 row mins
         col-min  = bf16 tensor_tensor min of each copy into a running
                    column accumulator (min over this core's rows)
Host: gathers row mins (dist1) and the 8 cores' [128,8192] column accumulators
(dist2 = min over cores x partitions), sqrt/mean/scale epilogue.
"""

import sys

sys.path.insert(0, "/opt/trn_rl_repo")

import numpy as np
import ml_dtypes

N = 8192           # points per cloud
NCORES = 8
NPC = N // NCORES  # 1024 rows per core
P = 128
BLKS = NPC // P    # 8 blocks
K = 24             # contraction rows
CH = 512           # matmul free dim (one PSUM bank fp32)
TW = 2048          # psum tile width (4 banks)
TPB = N // TW      # 4 psum tiles per block row sweep

_BUILT = None


def _limbs(x):
    h = x.astype(ml_dtypes.bfloat16).astype(np.float32)
    r = x - h
    m = r.astype(ml_dtypes.bfloat16).astype(np.float32)
    l = (r - m).astype(ml_dtypes.bfloat16).astype(np.float32)
    return h, m, l


def _stationary_rows(pts):
    """[24, n] lhsT rows: coord limbs + |p|^2 limbs + ones."""
    ph, pm, pl = _limbs(pts)
    p2 = np.sum(pts.astype(np.float64) ** 2, -1).astype(np.float32)
    p2h, p2m, p2l = _limbs(p2)
    one = np.ones_like(p2)
    return np.stack(
        [ph[:, 0], ph[:, 1], ph[:, 2],
         ph[:, 0], ph[:, 1], ph[:, 2],
         pm[:, 0], pm[:, 1], pm[:, 2],
         ph[:, 0], ph[:, 1], ph[:, 2],
         pl[:, 0], pl[:, 1], pl[:, 2],
         pm[:, 0], pm[:, 1], pm[:, 2],
         p2h, p2m, p2l,
         one, one, one], 0)


def _moving_rows(pts):
    """[24, n] rhs rows, limb-paired with _stationary_rows."""
    qh, qm, ql = _limbs(pts)
    q2 = np.sum(pts.astype(np.float64) ** 2, -1).astype(np.float32)
    q2h, q2m, q2l = _limbs(q2)
    one = np.ones_like(q2)
    return np.stack(
        [-2 * qh[:, 0], -2 * qh[:, 1], -2 * qh[:, 2],
         -2 * qm[:, 0], -2 * qm[:, 1], -2 * qm[:, 2],
         -2 * qh[:, 0], -2 * qh[:, 1], -2 * qh[:, 2],
         -2 * ql[:, 0], -2 * ql[:, 1], -2 * ql[:, 2],
         -2 * qh[:, 0], -2 * qh[:, 1], -2 * qh[:, 2],
         -2 * qm[:, 0], -2 * qm[:, 1], -2 * qm[:, 2],
         one, one, one,
         q2h, q2m, q2l], 0)


def _build():
    global _BUILT
    if _BUILT is not None:
        return _BUILT

    import concourse.bacc as bacc
    import concourse.mybir as mybir
    import concourse.tile as tile

    f32 = mybir.dt.float32
    bf16 = mybir.dt.bfloat16
    MIN = mybir.AluOpType.min
    X = mybir.AxisListType.X

    nc = bacc.Bacc(None, target_bir_lowering=False, debug=False)
    wts = nc.declare_dram_parameter("wts", [K, NPC], bf16, isOutput=False)
    rhs = nc.declare_dram_parameter("rhs", [K, N], bf16, isOutput=False)
    rowout_d = nc.declare_dram_parameter("rowout", [P, BLKS], f32, isOutput=True)
    colout_d = nc.declare_dram_parameter("colout", [P, N], bf16, isOutput=True)

    with tile.TileContext(nc) as tc:
        with tc.tile_pool(name="const", bufs=1) as cpool, \
             tc.tile_pool(name="cp", bufs=12) as cppool, \
             tc.tile_pool(name="ps", bufs=2, space="PSUM") as pspool:
            w_t = cpool.tile([K, NPC], bf16, name="w_t")
            r_t = cpool.tile([K, N], bf16, name="r_t")
            nc.sync.dma_start(out=w_t[:, :], in_=wts[:, :])
            nc.sync.dma_start(out=r_t[:, :], in_=rhs[:, :])

            # column accumulators, one per 2048-wide chunk group
            colacc = [cpool.tile([P, TW], bf16, name=f"colacc{cc}")
                      for cc in range(TPB)]
            s1 = cpool.tile([P, TW], bf16, name="s1")
            s2 = cpool.tile([P, TW], bf16, name="s2")
            rowout = cpool.tile([P, BLKS], f32, name="rowout")

            for b in range(BLKS):
                lhsT = w_t[:, b * P:(b + 1) * P]
                copies = []
                for cc in range(TPB):
                    pst = pspool.tile([P, TW], f32, name="pst")
                    for j in range(TW // CH):
                        c = cc * (TW // CH) + j
                        nc.tensor.matmul(
                            out=pst[:, j * CH:(j + 1) * CH],
                            lhsT=lhsT,
                            rhs=r_t[:, c * CH:(c + 1) * CH],
                        )
                    if b == 0:
                        cp_t = colacc[cc]     # block 0 copies ARE the init
                    else:
                        cp_t = cppool.tile([P, TW], bf16, name="cp", tag="cp")
                    nc.scalar.copy(out=cp_t[:, :], in_=pst[:, :])
                    copies.append(cp_t)

                # row-min tree over the block's 4 copies (bf16 2x TT mode)
                nc.vector.tensor_tensor(out=s1, in0=copies[0], in1=copies[1], op=MIN)
                nc.vector.tensor_tensor(out=s2, in0=copies[2], in1=copies[3], op=MIN)
                nc.vector.tensor_tensor(out=s1, in0=s1, in1=s2, op=MIN)
                nc.vector.tensor_tensor(
                    out=s2[:, 0:TW // 2], in0=s1[:, 0:TW // 2], in1=s1[:, TW // 2:TW], op=MIN)
                nc.vector.tensor_tensor(
                    out=s1[:, 0:TW // 4], in0=s2[:, 0:TW // 4], in1=s2[:, TW // 4:TW // 2], op=MIN)
                nc.vector.tensor_reduce(
                    out=rowout[:, b:b + 1], in_=s1[:, 0:TW // 4], axis=X, op=MIN)

                # column accumulation (skip block 0 — copies initialized it)
                if b > 0:
                    for cc in range(TPB):
                        nc.vector.tensor_tensor(
                            out=colacc[cc], in0=copies[cc], in1=colacc[cc], op=MIN)

            nc.sync.dma_start(out=rowout_d[:, :], in_=rowout[:, :])
            for cc in range(TPB):
                nc.sync.dma_start(
                    out=colout_d[:, cc * TW:(cc + 1) * TW], in_=colacc[cc][:, :])

    nc.compile()
    _BUILT = nc
    return nc


def kernel(target, output, cur, substeps):
    from concourse.bass_utils import run_bass_kernel_spmd

    a = np.asarray(target, dtype=np.float32)[0]   # (8192, 3) target cloud
    b = np.asarray(output, dtype=np.float32)[0]   # (8192, 3) output cloud
    cur = int(np.asarray(cur))
    substeps = int(np.asarray(substeps))

    bf = ml_dtypes.bfloat16
    w_full = _stationary_rows(a).astype(bf)       # [24, 8192] rows = targets
    r_full = _moving_rows(b).astype(bf)           # [24, 8192] cols = outputs

    in_maps = []
    for c in range(NCORES):
        sl = slice(c * NPC, (c + 1) * NPC)
        in_maps.append({"wts": np.ascontiguousarray(w_full[:, sl]),
                        "rhs": r_full})

    nc = _build()
    results = run_bass_kernel_spmd(nc, in_maps, list(range(NCORES))).results

    # dist1: per-target row minima (already min over all output points)
    d1 = np.concatenate(
        [results[c]["rowout"].T.reshape(-1) for c in range(NCORES)])  # (8192,)
    # dist2: fold cores and partitions of the column accumulators
    colmins = np.stack([
        results[c]["colout"].astype(np.float32).min(axis=0)
        for c in range(NCORES)])                                      # (8, 8192)
    d2 = colmins.min(axis=0)

    m1 = np.sqrt(np.maximum(d1, 0.0)).mean()
    m2 = np.sqrt(np.maximum(d2, 0.0)).mean()
    loss = 0.5 * (m1 + m2)
    scale = 10.0 / (0.99 ** (cur // substeps))
    return np.float32(loss * scale)


# revision 11
# speedup vs baseline: 1.6932x; 1.0080x over previous
"""Chamfer loss (nn_ChamferLoss) on 8 Trainium2 NeuronCores.

V2: one-matrix formulation. Each core computes its 1024-target-row stripe of
the 8192x8192 squared-distance matrix exactly once as K=24 bf16 matmuls (3-limb
bf16 decomposition of the fp32 operands -> fp32-accurate d2 at full PE rate).

Engine split per [128,2048] PSUM tile:
  PE   : 16 back-to-back matmuls per 128-row block (shared weights)
  ACT  : evacuates PSUM -> SBUF bf16 copies (scalar engine copy)
  DVE  : row-min  = bf16 tensor_tensor min tree over the block's 4 copies
                    (2x packed mode) + one small reduce  -> dist1 row mins
         col-min  = bf16 tensor_tensor min of each copy into a running
                    column accumulator (min over this core's rows)
Host: gathers row mins (dist1) and the 8 cores' [128,8192] column accumulators
(dist2 = min over cores x partitions), sqrt/mean/scale epilogue.
"""

import sys

sys.path.insert(0, "/opt/trn_rl_repo")

import numpy as np
import ml_dtypes

N = 8192           # points per cloud
NCORES = 8
NPC = N // NCORES  # 1024 rows per core
P = 128
BLKS = NPC // P    # 8 blocks
K = 24             # contraction rows
CH = 512           # matmul free dim (one PSUM bank fp32)
TW = 2048          # psum tile width (4 banks)
TPB = N // TW      # 4 psum tiles per block row sweep

_BUILT = None


def _limbs(x):
    h = x.astype(ml_dtypes.bfloat16).astype(np.float32)
    r = x - h
    m = r.astype(ml_dtypes.bfloat16).astype(np.float32)
    l = (r - m).astype(ml_dtypes.bfloat16).astype(np.float32)
    return h, m, l


def _stationary_rows(pts):
    """[24, n] lhsT rows: coord limbs + |p|^2 limbs + ones."""
    ph, pm, pl = _limbs(pts)
    p2 = np.sum(pts.astype(np.float64) ** 2, -1).astype(np.float32)
    p2h, p2m, p2l = _limbs(p2)
    one = np.ones_like(p2)
    return np.stack(
        [ph[:, 0], ph[:, 1], ph[:, 2],
         ph[:, 0], ph[:, 1], ph[:, 2],
         pm[:, 0], pm[:, 1], pm[:, 2],
         ph[:, 0], ph[:, 1], ph[:, 2],
         pl[:, 0], pl[:, 1], pl[:, 2],
         pm[:, 0], pm[:, 1], pm[:, 2],
         p2h, p2m, p2l,
         one, one, one], 0)


def _moving_rows(pts):
    """[24, n] rhs rows, limb-paired with _stationary_rows."""
    qh, qm, ql = _limbs(pts)
    q2 = np.sum(pts.astype(np.float64) ** 2, -1).astype(np.float32)
    q2h, q2m, q2l = _limbs(q2)
    one = np.ones_like(q2)
    return np.stack(
        [-2 * qh[:, 0], -2 * qh[:, 1], -2 * qh[:, 2],
         -2 * qm[:, 0], -2 * qm[:, 1], -2 * qm[:, 2],
         -2 * qh[:, 0], -2 * qh[:, 1], -2 * qh[:, 2],
         -2 * ql[:, 0], -2 * ql[:, 1], -2 * ql[:, 2],
         -2 * qh[:, 0], -2 * qh[:, 1], -2 * qh[:, 2],
         -2 * qm[:, 0], -2 * qm[:, 1], -2 * qm[:, 2],
         one, one, one,
         q2h, q2m, q2l], 0)


def _build():
    global _BUILT
    if _BUILT is not None:
        return _BUILT

    import concourse.bacc as bacc
    import concourse.mybir as mybir
    import concourse.tile as tile

    f32 = mybir.dt.float32
    bf16 = mybir.dt.bfloat16
    MIN = mybir.AluOpType.min
    X = mybir.AxisListType.X

    nc = bacc.Bacc(None, target_bir_lowering=False, debug=False)
    wts = nc.declare_dram_parameter("wts", [K, NPC], bf16, isOutput=False)
    rhs = nc.declare_dram_parameter("rhs", [K, N], bf16, isOutput=False)
    rowout_d = nc.declare_dram_parameter("rowout", [P, BLKS], f32, isOutput=True)
    colout_d = nc.declare_dram_parameter("colout", [P, N], bf16, isOutput=True)

    with tile.TileContext(nc) as tc:
        with tc.tile_pool(name="const", bufs=1) as cpool, \
             tc.tile_pool(name="cp", bufs=28) as cppool, \
             tc.tile_pool(name="ps", bufs=2, space="PSUM") as pspool:
            w_t = cpool.tile([K, NPC], bf16, name="w_t")
            r_t = cpool.tile([K, N], bf16, name="r_t")
            nc.sync.dma_start(out=w_t[:, :], in_=wts[:, :])
            nc.sync.dma_start(out=r_t[:, :], in_=rhs[:, :])

            # column accumulators, one per 2048-wide chunk group
            colacc = [cpool.tile([P, TW], bf16, name=f"colacc{cc}")
                      for cc in range(TPB)]
            s1 = cpool.tile([P, TW], bf16, name="s1")
            s2 = cpool.tile([P, TW], bf16, name="s2")
            rowfold = cpool.tile([P, BLKS * (TW // 4)], bf16, name="rowfold")
            rowout = cpool.tile([P, BLKS], f32, name="rowout")

            for b in range(BLKS):
                lhsT = w_t[:, b * P:(b + 1) * P]
                copies = []
                for cc in range(TPB):
                    pst = pspool.tile([P, TW], f32, name="pst")
                    for j in range(TW // CH):
                        c = cc * (TW // CH) + j
                        nc.tensor.matmul(
                            out=pst[:, j * CH:(j + 1) * CH],
                            lhsT=lhsT,
                            rhs=r_t[:, c * CH:(c + 1) * CH],
                        )
                    if b == 0:
                        cp_t = colacc[cc]     # block 0 copies ARE the init
                    else:
                        cp_t = cppool.tile([P, TW], bf16, name="cp", tag="cp")
                    nc.scalar.copy(out=cp_t[:, :], in_=pst[:, :])
                    copies.append(cp_t)

                # row-min tree over the block's 4 copies (bf16 2x TT mode)
                nc.vector.tensor_tensor(out=s1, in0=copies[0], in1=copies[1], op=MIN)
                nc.vector.tensor_tensor(out=s2, in0=copies[2], in1=copies[3], op=MIN)
                nc.vector.tensor_tensor(out=s1, in0=s1, in1=s2, op=MIN)
                nc.vector.tensor_tensor(
                    out=s2[:, 0:TW // 2], in0=s1[:, 0:TW // 2], in1=s1[:, TW // 2:TW], op=MIN)
                nc.vector.tensor_tensor(
                    out=rowfold[:, b * (TW // 4):(b + 1) * (TW // 4)],
                    in0=s2[:, 0:TW // 4], in1=s2[:, TW // 4:TW // 2], op=MIN)

                # column accumulation (skip block 0 — copies initialized it)
                if b > 0:
                    for cc in range(TPB):
                        nc.vector.tensor_tensor(
                            out=colacc[cc], in0=copies[cc], in1=colacc[cc], op=MIN)

            # one batched reduce over all 8 blocks' 512-wide row folds
            nc.vector.tensor_reduce(
                out=rowout[:, :],
                in_=rowfold.rearrange("p (b w) -> p b w", b=BLKS),
                axis=X, op=MIN)
            nc.sync.dma_start(out=rowout_d[:, :], in_=rowout[:, :])
            for cc in range(TPB):
                nc.sync.dma_start(
                    out=colout_d[:, cc * TW:(cc + 1) * TW], in_=colacc[cc][:, :])

    nc.compile()
    _BUILT = nc
    return nc


def kernel(target, output, cur, substeps):
    from concourse.bass_utils import run_bass_kernel_spmd

    a = np.asarray(target, dtype=np.float32)[0]   # (8192, 3) target cloud
    b = np.asarray(output, dtype=np.float32)[0]   # (8192, 3) output cloud
    cur = int(np.asarray(cur))
    substeps = int(np.asarray(substeps))

    bf = ml_dtypes.bfloat16
    w_full = _stationary_rows(a).astype(bf)       # [24, 8192] rows = targets
    r_full = _moving_rows(b).astype(bf)           # [24, 8192] cols = outputs

    in_maps = []
    for c in range(NCORES):
        sl = slice(c * NPC, (c + 1) * NPC)
        in_maps.append({"wts": np.ascontiguousarray(w_full[:, sl]),
                        "rhs": r_full})

    nc = _build()
    results = run_bass_kernel_spmd(nc, in_maps, list(range(NCORES))).results

    # dist1: per-target row minima (already min over all output points)
    d1 = np.concatenate(
        [results[c]["rowout"].T.reshape(-1) for c in range(NCORES)])  # (8192,)
    # dist2: fold cores and partitions of the column accumulators
    colmins = np.stack([
        results[c]["colout"].astype(np.float32).min(axis=0)
        for c in range(NCORES)])                                      # (8, 8192)
    d2 = colmins.min(axis=0)

    m1 = np.sqrt(np.maximum(d1, 0.0)).mean()
    m2 = np.sqrt(np.maximum(d2, 0.0)).mean()
    loss = 0.5 * (m1 + m2)
    scale = 10.0 / (0.99 ** (cur // substeps))
    return np.float32(loss * scale)


# revision 12
# speedup vs baseline: 4.0549x; 2.3948x over previous
"""Chamfer loss (nn_ChamferLoss) on 8 Trainium2 NeuronCores.

V3: rank-window pruned brute force.

Host sorts both clouds by x. Core c owns the 1024-target slab of sorted rank
[1024c, 1024c+1024) and scans it against the W=2048 output points nearest in
sorted rank (a window centered on the slab, clipped at the ends). For sorted
gaussian clouds the true nearest neighbour lies inside that window for all but
a handful of points; every point carries a certificate (row-min <= squared
x-gap to the uncovered region) checked on the host, and uncertified points are
recomputed exactly on the host (a few points, exact patch).

Distance tiles are computed on the PE as K=24 bf16 matmuls (3-limb bf16
decomposition of fp32 -> fp32-accurate d2 at full bf16 PE rate).  Per
[128,2048] PSUM tile: ACT evacuates to a bf16 SBUF copy; DVE folds the copy
for the row-min (2x-packed bf16 tensor_tensor min) and accumulates the
column-min across the 8 blocks.  dist1 row-mins and the per-core column-min
window go back to the host, which folds partitions/cores, applies the
certificates, patches, and finishes sqrt/mean/scale.
"""

import sys

sys.path.insert(0, "/opt/trn_rl_repo")

import numpy as np
import ml_dtypes

N = 8192           # points per cloud
NCORES = 8
NPC = N // NCORES  # 1024 targets per core
P = 128
BLKS = NPC // P    # 8 blocks per core
K = 24             # contraction rows (3-limb decomposition)
CH = 512           # matmul free dim (one PSUM bank fp32)
W = 2048           # output-point window per core
CERT_MARGIN = 1.01 # bf16 slack when checking certificates

_BUILT = None


def _limbs(x):
    h = x.astype(ml_dtypes.bfloat16).astype(np.float32)
    r = x - h
    m = r.astype(ml_dtypes.bfloat16).astype(np.float32)
    l = (r - m).astype(ml_dtypes.bfloat16).astype(np.float32)
    return h, m, l


def _stationary_rows(pts):
    """[24, n] lhsT rows: coord limbs + |p|^2 limbs + ones."""
    ph, pm, pl = _limbs(pts)
    p2 = np.sum(pts.astype(np.float64) ** 2, -1).astype(np.float32)
    p2h, p2m, p2l = _limbs(p2)
    one = np.ones_like(p2)
    return np.stack(
        [ph[:, 0], ph[:, 1], ph[:, 2],
         ph[:, 0], ph[:, 1], ph[:, 2],
         pm[:, 0], pm[:, 1], pm[:, 2],
         ph[:, 0], ph[:, 1], ph[:, 2],
         pl[:, 0], pl[:, 1], pl[:, 2],
         pm[:, 0], pm[:, 1], pm[:, 2],
         p2h, p2m, p2l,
         one, one, one], 0)


def _moving_rows(pts):
    """[24, n] rhs rows, limb-paired with _stationary_rows."""
    qh, qm, ql = _limbs(pts)
    q2 = np.sum(pts.astype(np.float64) ** 2, -1).astype(np.float32)
    q2h, q2m, q2l = _limbs(q2)
    one = np.ones_like(q2)
    return np.stack(
        [-2 * qh[:, 0], -2 * qh[:, 1], -2 * qh[:, 2],
         -2 * qm[:, 0], -2 * qm[:, 1], -2 * qm[:, 2],
         -2 * qh[:, 0], -2 * qh[:, 1], -2 * qh[:, 2],
         -2 * ql[:, 0], -2 * ql[:, 1], -2 * ql[:, 2],
         -2 * qh[:, 0], -2 * qh[:, 1], -2 * qh[:, 2],
         -2 * qm[:, 0], -2 * qm[:, 1], -2 * qm[:, 2],
         one, one, one,
         q2h, q2m, q2l], 0)


def _build():
    global _BUILT
    if _BUILT is not None:
        return _BUILT

    import concourse.bacc as bacc
    import concourse.mybir as mybir
    import concourse.tile as tile

    f32 = mybir.dt.float32
    bf16 = mybir.dt.bfloat16
    MIN = mybir.AluOpType.min
    X = mybir.AxisListType.X

    nc = bacc.Bacc(None, target_bir_lowering=False, debug=False)
    wts = nc.declare_dram_parameter("wts", [K, NPC], bf16, isOutput=False)
    rhs = nc.declare_dram_parameter("rhs", [K, W], bf16, isOutput=False)
    rowout_d = nc.declare_dram_parameter("rowout", [P, BLKS], f32, isOutput=True)
    colout_d = nc.declare_dram_parameter("colout", [P, W], bf16, isOutput=True)

    with tile.TileContext(nc) as tc:
        with tc.tile_pool(name="const", bufs=1) as cpool, \
             tc.tile_pool(name="cp", bufs=8) as cppool, \
             tc.tile_pool(name="ps", bufs=2, space="PSUM") as pspool:
            w_t = cpool.tile([K, NPC], bf16, name="w_t")
            r_t = cpool.tile([K, W], bf16, name="r_t")
            nc.sync.dma_start(out=w_t[:, :], in_=wts[:, :])
            nc.sync.dma_start(out=r_t[:, :], in_=rhs[:, :])

            colacc = cpool.tile([P, W], bf16, name="colacc")
            s1 = cpool.tile([P, W // 2], bf16, name="s1")
            s2 = cpool.tile([P, W // 4], bf16, name="s2")
            rowout = cpool.tile([P, BLKS], f32, name="rowout")

            for b in range(BLKS):
                lhsT = w_t[:, b * P:(b + 1) * P]
                pst = pspool.tile([P, W], f32, name="pst")
                for j in range(W // CH):
                    nc.tensor.matmul(
                        out=pst[:, j * CH:(j + 1) * CH],
                        lhsT=lhsT,
                        rhs=r_t[:, j * CH:(j + 1) * CH],
                    )
                if b == 0:
                    cp_t = colacc                 # block 0 copy IS the init
                else:
                    cp_t = cppool.tile([P, W], bf16, name="cp", tag="cp")
                nc.scalar.copy(out=cp_t[:, :], in_=pst[:, :])

                # row-min: fold the copy 2048 -> 1024 -> 512, then reduce
                nc.vector.tensor_tensor(
                    out=s1, in0=cp_t[:, 0:W // 2], in1=cp_t[:, W // 2:W], op=MIN)
                nc.vector.tensor_tensor(
                    out=s2, in0=s1[:, 0:W // 4], in1=s1[:, W // 4:W // 2], op=MIN)
                nc.vector.tensor_reduce(
                    out=rowout[:, b:b + 1], in_=s2, axis=X, op=MIN)

                # column accumulation (block 0 initialized colacc directly)
                if b > 0:
                    nc.vector.tensor_tensor(
                        out=colacc, in0=cp_t, in1=colacc, op=MIN)

            nc.sync.dma_start(out=rowout_d[:, :], in_=rowout[:, :])
            nc.sync.dma_start(out=colout_d[:, :], in_=colacc[:, :])

    nc.compile()
    _BUILT = nc
    return nc


def _window(c):
    center = c * NPC + NPC // 2
    lo = min(max(0, center - W // 2), N - W)
    return lo, lo + W


def kernel(target, output, cur, substeps):
    from concourse.bass_utils import run_bass_kernel_spmd

    a = np.asarray(target, dtype=np.float32)[0]   # (8192,3) target cloud
    b = np.asarray(output, dtype=np.float32)[0]   # (8192,3) output cloud
    cur = int(np.asarray(cur))
    substeps = int(np.asarray(substeps))

    sa = np.argsort(a[:, 0], kind="stable")
    sb = np.argsort(b[:, 0], kind="stable")
    A = a[sa]                                     # sorted targets
    B = b[sb]                                     # sorted outputs

    bf = ml_dtypes.bfloat16
    w_full = _stationary_rows(A).astype(bf)       # [24, 8192]
    r_full = _moving_rows(B).astype(bf)           # [24, 8192]

    in_maps = []
    for c in range(NCORES):
        lo, hi = _window(c)
        in_maps.append({
            "wts": np.ascontiguousarray(w_full[:, c * NPC:(c + 1) * NPC]),
            "rhs": np.ascontiguousarray(r_full[:, lo:hi]),
        })

    nc = _build()
    results = run_bass_kernel_spmd(nc, in_maps, list(range(NCORES))).results

    A64 = A.astype(np.float64)
    B64 = B.astype(np.float64)
    a2 = np.sum(A64 ** 2, 1)
    b2 = np.sum(B64 ** 2, 1)

    # ---- dist1 (per sorted target) ----
    d1 = np.empty(N, np.float64)
    col_parts = []
    for c in range(NCORES):
        d1[c * NPC:(c + 1) * NPC] = results[c]["rowout"].T.reshape(-1)
        col_parts.append(results[c]["colout"].astype(np.float32).min(axis=0))

    # dist1 certificates: squared x-gap to the uncovered ranks
    bad1 = []
    for c in range(NCORES):
        lo, hi = _window(c)
        t = slice(c * NPC, (c + 1) * NPC)
        gl = (A[t, 0] - B[lo - 1, 0]) ** 2 if lo > 0 else np.full(NPC, np.inf)
        gr = (B[hi, 0] - A[t, 0]) ** 2 if hi < N else np.full(NPC, np.inf)
        fail = d1[t] * CERT_MARGIN > np.minimum(gl, gr)
        bad1.extend((c * NPC + np.nonzero(fail)[0]).tolist())
    for t in bad1:
        d1[t] = np.min(a2[t] + b2 - 2.0 * (B64 @ A64[t]))

    # ---- dist2 (per sorted output) ----
    d2 = np.full(N, np.inf, np.float64)
    cov_lo = np.full(N, N, np.int64)
    cov_hi = np.zeros(N, np.int64)
    for c in range(NCORES):
        lo, hi = _window(c)
        np.minimum.at(d2, np.arange(lo, hi), col_parts[c].astype(np.float64))
        cov_lo[lo:hi] = np.minimum(cov_lo[lo:hi], c * NPC)
        cov_hi[lo:hi] = np.maximum(cov_hi[lo:hi], (c + 1) * NPC)
    gl = np.where(cov_lo > 0, (B[:, 0] - A[np.maximum(cov_lo - 1, 0), 0]) ** 2, np.inf)
    gr = np.where(cov_hi < N, (A[np.minimum(cov_hi, N - 1), 0] - B[:, 0]) ** 2, np.inf)
    bad2 = np.nonzero(d2 * CERT_MARGIN > np.minimum(gl, gr))[0]
    for j in bad2:
        d2[j] = np.min(b2[j] + a2 - 2.0 * (A64 @ B64[j]))

    m1 = np.sqrt(np.maximum(d1, 0.0)).mean()
    m2 = np.sqrt(np.maximum(d2, 0.0)).mean()
    loss = 0.5 * (m1 + m2)
    scale = 10.0 / (0.99 ** (cur // substeps))
    return np.float32(loss * scale)


# revision 13
# speedup vs baseline: 4.7325x; 1.1671x over previous
"""Chamfer loss (nn_ChamferLoss) on 8 Trainium2 NeuronCores.

V3: rank-window pruned brute force.

Host sorts both clouds by x. Core c owns the 1024-target slab of sorted rank
[1024c, 1024c+1024) and scans it against the W=2048 output points nearest in
sorted rank (a window centered on the slab, clipped at the ends). For sorted
gaussian clouds the true nearest neighbour lies inside that window for all but
a handful of points; every point carries a certificate (row-min <= squared
x-gap to the uncovered region) checked on the host, and uncertified points are
recomputed exactly on the host (a few points, exact patch).

Distance tiles are computed on the PE as K=24 bf16 matmuls (3-limb bf16
decomposition of fp32 -> fp32-accurate d2 at full bf16 PE rate).  Per
[128,2048] PSUM tile: ACT evacuates to a bf16 SBUF copy; DVE folds the copy
for the row-min (2x-packed bf16 tensor_tensor min) and accumulates the
column-min across the 8 blocks.  dist1 row-mins and the per-core column-min
window go back to the host, which folds partitions/cores, applies the
certificates, patches, and finishes sqrt/mean/scale.
"""

import sys

sys.path.insert(0, "/opt/trn_rl_repo")

import numpy as np
import ml_dtypes

N = 8192           # points per cloud
NCORES = 8
NPC = N // NCORES  # 1024 targets per core
P = 128
BLKS = NPC // P    # 8 blocks per core
K = 24             # contraction rows (3-limb decomposition)
CH = 512           # matmul free dim (one PSUM bank fp32)
W = 1536           # output-point window per core
CERT_MARGIN = 1.01 # bf16 slack when checking certificates

_BUILT = None


def _limbs(x):
    h = x.astype(ml_dtypes.bfloat16).astype(np.float32)
    r = x - h
    m = r.astype(ml_dtypes.bfloat16).astype(np.float32)
    l = (r - m).astype(ml_dtypes.bfloat16).astype(np.float32)
    return h, m, l


def _stationary_rows(pts):
    """[24, n] lhsT rows: coord limbs + |p|^2 limbs + ones."""
    ph, pm, pl = _limbs(pts)
    p2 = np.sum(pts.astype(np.float64) ** 2, -1).astype(np.float32)
    p2h, p2m, p2l = _limbs(p2)
    one = np.ones_like(p2)
    return np.stack(
        [ph[:, 0], ph[:, 1], ph[:, 2],
         ph[:, 0], ph[:, 1], ph[:, 2],
         pm[:, 0], pm[:, 1], pm[:, 2],
         ph[:, 0], ph[:, 1], ph[:, 2],
         pl[:, 0], pl[:, 1], pl[:, 2],
         pm[:, 0], pm[:, 1], pm[:, 2],
         p2h, p2m, p2l,
         one, one, one], 0)


def _moving_rows(pts):
    """[24, n] rhs rows, limb-paired with _stationary_rows."""
    qh, qm, ql = _limbs(pts)
    q2 = np.sum(pts.astype(np.float64) ** 2, -1).astype(np.float32)
    q2h, q2m, q2l = _limbs(q2)
    one = np.ones_like(q2)
    return np.stack(
        [-2 * qh[:, 0], -2 * qh[:, 1], -2 * qh[:, 2],
         -2 * qm[:, 0], -2 * qm[:, 1], -2 * qm[:, 2],
         -2 * qh[:, 0], -2 * qh[:, 1], -2 * qh[:, 2],
         -2 * ql[:, 0], -2 * ql[:, 1], -2 * ql[:, 2],
         -2 * qh[:, 0], -2 * qh[:, 1], -2 * qh[:, 2],
         -2 * qm[:, 0], -2 * qm[:, 1], -2 * qm[:, 2],
         one, one, one,
         q2h, q2m, q2l], 0)


def _build():
    global _BUILT
    if _BUILT is not None:
        return _BUILT

    import concourse.bacc as bacc
    import concourse.mybir as mybir
    import concourse.tile as tile

    f32 = mybir.dt.float32
    bf16 = mybir.dt.bfloat16
    MIN = mybir.AluOpType.min
    X = mybir.AxisListType.X

    nc = bacc.Bacc(None, target_bir_lowering=False, debug=False)
    wts = nc.declare_dram_parameter("wts", [K, NPC], bf16, isOutput=False)
    rhs = nc.declare_dram_parameter("rhs", [K, W], bf16, isOutput=False)
    rowout_d = nc.declare_dram_parameter("rowout", [P, BLKS], f32, isOutput=True)
    colout_d = nc.declare_dram_parameter("colout", [P, W], bf16, isOutput=True)

    with tile.TileContext(nc) as tc:
        with tc.tile_pool(name="const", bufs=1) as cpool, \
             tc.tile_pool(name="cp", bufs=8) as cppool, \
             tc.tile_pool(name="ps", bufs=2, space="PSUM") as pspool:
            w_t = cpool.tile([K, NPC], bf16, name="w_t")
            r_t = cpool.tile([K, W], bf16, name="r_t")
            nc.sync.dma_start(out=w_t[:, :], in_=wts[:, :])
            nc.sync.dma_start(out=r_t[:, :], in_=rhs[:, :])

            colacc = cpool.tile([P, W], bf16, name="colacc")
            s1 = cpool.tile([P, W // 2], bf16, name="s1")
            s2 = cpool.tile([P, W // 4], bf16, name="s2")
            rowout = cpool.tile([P, BLKS], f32, name="rowout")

            for b in range(BLKS):
                lhsT = w_t[:, b * P:(b + 1) * P]
                pst = pspool.tile([P, W], f32, name="pst")
                for j in range(W // CH):
                    nc.tensor.matmul(
                        out=pst[:, j * CH:(j + 1) * CH],
                        lhsT=lhsT,
                        rhs=r_t[:, j * CH:(j + 1) * CH],
                    )
                if b == 0:
                    cp_t = colacc                 # block 0 copy IS the init
                else:
                    cp_t = cppool.tile([P, W], bf16, name="cp", tag="cp")
                nc.scalar.copy(out=cp_t[:, :], in_=pst[:, :])

                # row-min: fold the copy 2048 -> 1024 -> 512, then reduce
                nc.vector.tensor_tensor(
                    out=s1, in0=cp_t[:, 0:W // 2], in1=cp_t[:, W // 2:W], op=MIN)
                nc.vector.tensor_tensor(
                    out=s2, in0=s1[:, 0:W // 4], in1=s1[:, W // 4:W // 2], op=MIN)
                nc.vector.tensor_reduce(
                    out=rowout[:, b:b + 1], in_=s2, axis=X, op=MIN)

                # column accumulation (block 0 initialized colacc directly)
                if b > 0:
                    nc.vector.tensor_tensor(
                        out=colacc, in0=cp_t, in1=colacc, op=MIN)

            nc.sync.dma_start(out=rowout_d[:, :], in_=rowout[:, :])
            nc.sync.dma_start(out=colout_d[:, :], in_=colacc[:, :])

    nc.compile()
    _BUILT = nc
    return nc


def _window(c):
    center = c * NPC + NPC // 2
    lo = min(max(0, center - W // 2), N - W)
    return lo, lo + W


def kernel(target, output, cur, substeps):
    from concourse.bass_utils import run_bass_kernel_spmd

    a = np.asarray(target, dtype=np.float32)[0]   # (8192,3) target cloud
    b = np.asarray(output, dtype=np.float32)[0]   # (8192,3) output cloud
    cur = int(np.asarray(cur))
    substeps = int(np.asarray(substeps))

    sa = np.argsort(a[:, 0], kind="stable")
    sb = np.argsort(b[:, 0], kind="stable")
    A = a[sa]                                     # sorted targets
    B = b[sb]                                     # sorted outputs

    bf = ml_dtypes.bfloat16
    w_full = _stationary_rows(A).astype(bf)       # [24, 8192]
    r_full = _moving_rows(B).astype(bf)           # [24, 8192]

    in_maps = []
    for c in range(NCORES):
        lo, hi = _window(c)
        in_maps.append({
            "wts": np.ascontiguousarray(w_full[:, c * NPC:(c + 1) * NPC]),
            "rhs": np.ascontiguousarray(r_full[:, lo:hi]),
        })

    nc = _build()
    results = run_bass_kernel_spmd(nc, in_maps, list(range(NCORES))).results

    A64 = A.astype(np.float64)
    B64 = B.astype(np.float64)
    a2 = np.sum(A64 ** 2, 1)
    b2 = np.sum(B64 ** 2, 1)

    # ---- dist1 (per sorted target) ----
    d1 = np.empty(N, np.float64)
    col_parts = []
    for c in range(NCORES):
        d1[c * NPC:(c + 1) * NPC] = results[c]["rowout"].T.reshape(-1)
        col_parts.append(results[c]["colout"].astype(np.float32).min(axis=0))

    # dist1 certificates: squared x-gap to the uncovered ranks
    bad1 = []
    for c in range(NCORES):
        lo, hi = _window(c)
        t = slice(c * NPC, (c + 1) * NPC)
        gl = (A[t, 0] - B[lo - 1, 0]) ** 2 if lo > 0 else np.full(NPC, np.inf)
        gr = (B[hi, 0] - A[t, 0]) ** 2 if hi < N else np.full(NPC, np.inf)
        fail = d1[t] * CERT_MARGIN > np.minimum(gl, gr)
        bad1.extend((c * NPC + np.nonzero(fail)[0]).tolist())
    for t in bad1:
        d1[t] = np.min(a2[t] + b2 - 2.0 * (B64 @ A64[t]))

    # ---- dist2 (per sorted output) ----
    d2 = np.full(N, np.inf, np.float64)
    cov_lo = np.full(N, N, np.int64)
    cov_hi = np.zeros(N, np.int64)
    for c in range(NCORES):
        lo, hi = _window(c)
        np.minimum.at(d2, np.arange(lo, hi), col_parts[c].astype(np.float64))
        cov_lo[lo:hi] = np.minimum(cov_lo[lo:hi], c * NPC)
        cov_hi[lo:hi] = np.maximum(cov_hi[lo:hi], (c + 1) * NPC)
    gl = np.where(cov_lo > 0, (B[:, 0] - A[np.maximum(cov_lo - 1, 0), 0]) ** 2, np.inf)
    gr = np.where(cov_hi < N, (A[np.minimum(cov_hi, N - 1), 0] - B[:, 0]) ** 2, np.inf)
    bad2 = np.nonzero(d2 * CERT_MARGIN > np.minimum(gl, gr))[0]
    for j in bad2:
        d2[j] = np.min(b2[j] + a2 - 2.0 * (A64 @ B64[j]))

    m1 = np.sqrt(np.maximum(d1, 0.0)).mean()
    m2 = np.sqrt(np.maximum(d2, 0.0)).mean()
    loss = 0.5 * (m1 + m2)
    scale = 10.0 / (0.99 ** (cur // substeps))
    return np.float32(loss * scale)


# revision 15
# speedup vs baseline: 4.7695x; 1.0078x over previous
"""Chamfer loss (nn_ChamferLoss) on 8 Trainium2 NeuronCores.

V3: rank-window pruned brute force.

Host sorts both clouds by x. Core c owns the 1024-target slab of sorted rank
[1024c, 1024c+1024) and scans it against the W=2048 output points nearest in
sorted rank (a window centered on the slab, clipped at the ends). For sorted
gaussian clouds the true nearest neighbour lies inside that window for all but
a handful of points; every point carries a certificate (row-min <= squared
x-gap to the uncovered region) checked on the host, and uncertified points are
recomputed exactly on the host (a few points, exact patch).

Distance tiles are computed on the PE as K=24 bf16 matmuls (3-limb bf16
decomposition of fp32 -> fp32-accurate d2 at full bf16 PE rate).  Per
[128,2048] PSUM tile: ACT evacuates to a bf16 SBUF copy; DVE folds the copy
for the row-min (2x-packed bf16 tensor_tensor min) and accumulates the
column-min across the 8 blocks.  dist1 row-mins and the per-core column-min
window go back to the host, which folds partitions/cores, applies the
certificates, patches, and finishes sqrt/mean/scale.
"""

import sys

sys.path.insert(0, "/opt/trn_rl_repo")

import numpy as np
import ml_dtypes

N = 8192           # points per cloud
NCORES = 8
NPC = N // NCORES  # 1024 targets per core
P = 128
BLKS = NPC // P    # 8 blocks per core
K = 24             # contraction rows (3-limb decomposition)
CH = 512           # matmul free dim (one PSUM bank fp32)
W = 1536           # output-point window per core
CERT_MARGIN = 1.01 # bf16 slack when checking certificates

_BUILT = None


def _limbs(x):
    h = x.astype(ml_dtypes.bfloat16).astype(np.float32)
    r = x - h
    m = r.astype(ml_dtypes.bfloat16).astype(np.float32)
    l = (r - m).astype(ml_dtypes.bfloat16).astype(np.float32)
    return h, m, l


def _stationary_rows(pts):
    """[24, n] lhsT rows: coord limbs + |p|^2 limbs + ones."""
    ph, pm, pl = _limbs(pts)
    p2 = np.sum(pts.astype(np.float64) ** 2, -1).astype(np.float32)
    p2h, p2m, p2l = _limbs(p2)
    one = np.ones_like(p2)
    return np.stack(
        [ph[:, 0], ph[:, 1], ph[:, 2],
         ph[:, 0], ph[:, 1], ph[:, 2],
         pm[:, 0], pm[:, 1], pm[:, 2],
         ph[:, 0], ph[:, 1], ph[:, 2],
         pl[:, 0], pl[:, 1], pl[:, 2],
         pm[:, 0], pm[:, 1], pm[:, 2],
         p2h, p2m, p2l,
         one, one, one], 0)


def _moving_rows(pts):
    """[24, n] rhs rows, limb-paired with _stationary_rows."""
    qh, qm, ql = _limbs(pts)
    q2 = np.sum(pts.astype(np.float64) ** 2, -1).astype(np.float32)
    q2h, q2m, q2l = _limbs(q2)
    one = np.ones_like(q2)
    return np.stack(
        [-2 * qh[:, 0], -2 * qh[:, 1], -2 * qh[:, 2],
         -2 * qm[:, 0], -2 * qm[:, 1], -2 * qm[:, 2],
         -2 * qh[:, 0], -2 * qh[:, 1], -2 * qh[:, 2],
         -2 * ql[:, 0], -2 * ql[:, 1], -2 * ql[:, 2],
         -2 * qh[:, 0], -2 * qh[:, 1], -2 * qh[:, 2],
         -2 * qm[:, 0], -2 * qm[:, 1], -2 * qm[:, 2],
         one, one, one,
         q2h, q2m, q2l], 0)


def _build():
    global _BUILT
    if _BUILT is not None:
        return _BUILT

    import concourse.bacc as bacc
    import concourse.mybir as mybir
    import concourse.tile as tile

    f32 = mybir.dt.float32
    bf16 = mybir.dt.bfloat16
    MIN = mybir.AluOpType.min
    X = mybir.AxisListType.X

    nc = bacc.Bacc(None, target_bir_lowering=False, debug=False)
    wts = nc.declare_dram_parameter("wts", [K, NPC], bf16, isOutput=False)
    rhs = nc.declare_dram_parameter("rhs", [K, W], bf16, isOutput=False)
    rowout_d = nc.declare_dram_parameter("rowout", [P, BLKS], f32, isOutput=True)
    colout_d = nc.declare_dram_parameter("colout", [P, W], bf16, isOutput=True)

    with tile.TileContext(nc) as tc:
        with tc.tile_pool(name="const", bufs=1) as cpool, \
             tc.tile_pool(name="cp", bufs=8) as cppool, \
             tc.tile_pool(name="ps", bufs=2, space="PSUM") as pspool:
            w_t = cpool.tile([K, NPC], bf16, name="w_t")
            r_t = cpool.tile([K, W], bf16, name="r_t")
            # parallel queues + chunked rhs so block 0's matmuls start early
            nc.gpsimd.dma_start(out=w_t[:, :], in_=wts[:, :])
            for j in range(W // CH):
                nc.sync.dma_start(
                    out=r_t[:, j * CH:(j + 1) * CH], in_=rhs[:, j * CH:(j + 1) * CH])

            colacc = cpool.tile([P, W], bf16, name="colacc")
            s1 = cpool.tile([P, W // 2], bf16, name="s1")
            s2 = cpool.tile([P, W // 4], bf16, name="s2")
            rowout = cpool.tile([P, BLKS], f32, name="rowout")

            for b in range(BLKS):
                lhsT = w_t[:, b * P:(b + 1) * P]
                pst = pspool.tile([P, W], f32, name="pst")
                for j in range(W // CH):
                    nc.tensor.matmul(
                        out=pst[:, j * CH:(j + 1) * CH],
                        lhsT=lhsT,
                        rhs=r_t[:, j * CH:(j + 1) * CH],
                    )
                if b == 0:
                    cp_t = colacc                 # block 0 copy IS the init
                else:
                    cp_t = cppool.tile([P, W], bf16, name="cp", tag="cp")
                nc.scalar.copy(out=cp_t[:, :], in_=pst[:, :])

                # column accumulation first (feeds the final colout DMA)
                if b > 0:
                    nc.vector.tensor_tensor(
                        out=colacc, in0=cp_t, in1=colacc, op=MIN)

                # row-min: fold the copy W -> W/2 -> W/4, then reduce
                nc.vector.tensor_tensor(
                    out=s1, in0=cp_t[:, 0:W // 2], in1=cp_t[:, W // 2:W], op=MIN)
                nc.vector.tensor_tensor(
                    out=s2, in0=s1[:, 0:W // 4], in1=s1[:, W // 4:W // 2], op=MIN)
                nc.vector.tensor_reduce(
                    out=rowout[:, b:b + 1], in_=s2, axis=X, op=MIN)

            nc.sync.dma_start(out=rowout_d[:, :], in_=rowout[:, :])
            nc.sync.dma_start(out=colout_d[:, :], in_=colacc[:, :])

    nc.compile()
    _BUILT = nc
    return nc


def _window(c):
    center = c * NPC + NPC // 2
    lo = min(max(0, center - W // 2), N - W)
    return lo, lo + W


def kernel(target, output, cur, substeps):
    from concourse.bass_utils import run_bass_kernel_spmd

    a = np.asarray(target, dtype=np.float32)[0]   # (8192,3) target cloud
    b = np.asarray(output, dtype=np.float32)[0]   # (8192,3) output cloud
    cur = int(np.asarray(cur))
    substeps = int(np.asarray(substeps))

    sa = np.argsort(a[:, 0], kind="stable")
    sb = np.argsort(b[:, 0], kind="stable")
    A = a[sa]                                     # sorted targets
    B = b[sb]                                     # sorted outputs

    bf = ml_dtypes.bfloat16
    w_full = _stationary_rows(A).astype(bf)       # [24, 8192]
    r_full = _moving_rows(B).astype(bf)           # [24, 8192]

    in_maps = []
    for c in range(NCORES):
        lo, hi = _window(c)
        in_maps.append({
            "wts": np.ascontiguousarray(w_full[:, c * NPC:(c + 1) * NPC]),
            "rhs": np.ascontiguousarray(r_full[:, lo:hi]),
        })

    nc = _build()
    results = run_bass_kernel_spmd(nc, in_maps, list(range(NCORES))).results

    A64 = A.astype(np.float64)
    B64 = B.astype(np.float64)
    a2 = np.sum(A64 ** 2, 1)
    b2 = np.sum(B64 ** 2, 1)

    # ---- dist1 (per sorted target) ----
    d1 = np.empty(N, np.float64)
    col_parts = []
    for c in range(NCORES):
        d1[c * NPC:(c + 1) * NPC] = results[c]["rowout"].T.reshape(-1)
        col_parts.append(results[c]["colout"].astype(np.float32).min(axis=0))

    # dist1 certificates: squared x-gap to the uncovered ranks
    bad1 = []
    for c in range(NCORES):
        lo, hi = _window(c)
        t = slice(c * NPC, (c + 1) * NPC)
        gl = (A[t, 0] - B[lo - 1, 0]) ** 2 if lo > 0 else np.full(NPC, np.inf)
        gr = (B[hi, 0] - A[t, 0]) ** 2 if hi < N else np.full(NPC, np.inf)
        fail = d1[t] * CERT_MARGIN > np.minimum(gl, gr)
        bad1.extend((c * NPC + np.nonzero(fail)[0]).tolist())
    for t in bad1:
        d1[t] = np.min(a2[t] + b2 - 2.0 * (B64 @ A64[t]))

    # ---- dist2 (per sorted output) ----
    d2 = np.full(N, np.inf, np.float64)
    cov_lo = np.full(N, N, np.int64)
    cov_hi = np.zeros(N, np.int64)
    for c in range(NCORES):
        lo, hi = _window(c)
        np.minimum.at(d2, np.arange(lo, hi), col_parts[c].astype(np.float64))
        cov_lo[lo:hi] = np.minimum(cov_lo[lo:hi], c * NPC)
        cov_hi[lo:hi] = np.maximum(cov_hi[lo:hi], (c + 1) * NPC)
    gl = np.where(cov_lo > 0, (B[:, 0] - A[np.maximum(cov_lo - 1, 0), 0]) ** 2, np.inf)
    gr = np.where(cov_hi < N, (A[np.minimum(cov_hi, N - 1), 0] - B[:, 0]) ** 2, np.inf)
    bad2 = np.nonzero(d2 * CERT_MARGIN > np.minimum(gl, gr))[0]
    for j in bad2:
        d2[j] = np.min(b2[j] + a2 - 2.0 * (A64 @ B64[j]))

    m1 = np.sqrt(np.maximum(d1, 0.0)).mean()
    m2 = np.sqrt(np.maximum(d2, 0.0)).mean()
    loss = 0.5 * (m1 + m2)
    scale = 10.0 / (0.99 ** (cur // substeps))
    return np.float32(loss * scale)


# revision 16
# speedup vs baseline: 5.7444x; 1.2044x over previous
"""Chamfer loss (nn_ChamferLoss) on 8 Trainium2 NeuronCores.

V3: rank-window pruned brute force.

Host sorts both clouds by x. Core c owns the 1024-target slab of sorted rank
[1024c, 1024c+1024) and scans it against the W=2048 output points nearest in
sorted rank (a window centered on the slab, clipped at the ends). For sorted
gaussian clouds the true nearest neighbour lies inside that window for all but
a handful of points; every point carries a certificate (row-min <= squared
x-gap to the uncovered region) checked on the host, and uncertified points are
recomputed exactly on the host (a few points, exact patch).

Distance tiles are computed on the PE as K=24 bf16 matmuls (3-limb bf16
decomposition of fp32 -> fp32-accurate d2 at full bf16 PE rate).  Per
[128,2048] PSUM tile: ACT evacuates to a bf16 SBUF copy; DVE folds the copy
for the row-min (2x-packed bf16 tensor_tensor min) and accumulates the
column-min across the 8 blocks.  dist1 row-mins and the per-core column-min
window go back to the host, which folds partitions/cores, applies the
certificates, patches, and finishes sqrt/mean/scale.
"""

import sys

sys.path.insert(0, "/opt/trn_rl_repo")

import numpy as np
import ml_dtypes

N = 8192           # points per cloud
NCORES = 8
NPC = N // NCORES  # 1024 targets per core
P = 128
BLKS = NPC // P    # 8 blocks per core
K = 24             # contraction rows (3-limb decomposition)
CH = 512           # matmul free dim (one PSUM bank fp32)
W = 1024           # output-point window per core
CERT_MARGIN = 1.01 # bf16 slack when checking certificates

_BUILT = None


def _limbs(x):
    h = x.astype(ml_dtypes.bfloat16).astype(np.float32)
    r = x - h
    m = r.astype(ml_dtypes.bfloat16).astype(np.float32)
    l = (r - m).astype(ml_dtypes.bfloat16).astype(np.float32)
    return h, m, l


def _stationary_rows(pts):
    """[24, n] lhsT rows: coord limbs + |p|^2 limbs + ones."""
    ph, pm, pl = _limbs(pts)
    p2 = np.sum(pts.astype(np.float64) ** 2, -1).astype(np.float32)
    p2h, p2m, p2l = _limbs(p2)
    one = np.ones_like(p2)
    return np.stack(
        [ph[:, 0], ph[:, 1], ph[:, 2],
         ph[:, 0], ph[:, 1], ph[:, 2],
         pm[:, 0], pm[:, 1], pm[:, 2],
         ph[:, 0], ph[:, 1], ph[:, 2],
         pl[:, 0], pl[:, 1], pl[:, 2],
         pm[:, 0], pm[:, 1], pm[:, 2],
         p2h, p2m, p2l,
         one, one, one], 0)


def _moving_rows(pts):
    """[24, n] rhs rows, limb-paired with _stationary_rows."""
    qh, qm, ql = _limbs(pts)
    q2 = np.sum(pts.astype(np.float64) ** 2, -1).astype(np.float32)
    q2h, q2m, q2l = _limbs(q2)
    one = np.ones_like(q2)
    return np.stack(
        [-2 * qh[:, 0], -2 * qh[:, 1], -2 * qh[:, 2],
         -2 * qm[:, 0], -2 * qm[:, 1], -2 * qm[:, 2],
         -2 * qh[:, 0], -2 * qh[:, 1], -2 * qh[:, 2],
         -2 * ql[:, 0], -2 * ql[:, 1], -2 * ql[:, 2],
         -2 * qh[:, 0], -2 * qh[:, 1], -2 * qh[:, 2],
         -2 * qm[:, 0], -2 * qm[:, 1], -2 * qm[:, 2],
         one, one, one,
         q2h, q2m, q2l], 0)


def _build():
    global _BUILT
    if _BUILT is not None:
        return _BUILT

    import concourse.bacc as bacc
    import concourse.mybir as mybir
    import concourse.tile as tile

    f32 = mybir.dt.float32
    bf16 = mybir.dt.bfloat16
    MIN = mybir.AluOpType.min
    X = mybir.AxisListType.X

    nc = bacc.Bacc(None, target_bir_lowering=False, debug=False)
    wts = nc.declare_dram_parameter("wts", [K, NPC], bf16, isOutput=False)
    rhs = nc.declare_dram_parameter("rhs", [K, W], bf16, isOutput=False)
    rowout_d = nc.declare_dram_parameter("rowout", [P, BLKS], f32, isOutput=True)
    colout_d = nc.declare_dram_parameter("colout", [P, W], bf16, isOutput=True)

    with tile.TileContext(nc) as tc:
        with tc.tile_pool(name="const", bufs=1) as cpool, \
             tc.tile_pool(name="cp", bufs=8) as cppool, \
             tc.tile_pool(name="ps", bufs=2, space="PSUM") as pspool:
            w_t = cpool.tile([K, NPC], bf16, name="w_t")
            r_t = cpool.tile([K, W], bf16, name="r_t")
            # parallel queues + chunked rhs so block 0's matmuls start early
            nc.gpsimd.dma_start(out=w_t[:, :], in_=wts[:, :])
            for j in range(W // CH):
                nc.sync.dma_start(
                    out=r_t[:, j * CH:(j + 1) * CH], in_=rhs[:, j * CH:(j + 1) * CH])

            colacc = cpool.tile([P, W], bf16, name="colacc")
            s1 = cpool.tile([P, W // 2], bf16, name="s1")
            s2 = cpool.tile([P, W // 4], bf16, name="s2")
            rowout = cpool.tile([P, BLKS], f32, name="rowout")

            for b in range(BLKS):
                lhsT = w_t[:, b * P:(b + 1) * P]
                pst = pspool.tile([P, W], f32, name="pst")
                for j in range(W // CH):
                    nc.tensor.matmul(
                        out=pst[:, j * CH:(j + 1) * CH],
                        lhsT=lhsT,
                        rhs=r_t[:, j * CH:(j + 1) * CH],
                    )
                if b == 0:
                    cp_t = colacc                 # block 0 copy IS the init
                else:
                    cp_t = cppool.tile([P, W], bf16, name="cp", tag="cp")
                nc.scalar.copy(out=cp_t[:, :], in_=pst[:, :])

                # column accumulation first (feeds the final colout DMA)
                if b > 0:
                    nc.vector.tensor_tensor(
                        out=colacc, in0=cp_t, in1=colacc, op=MIN)

                # row-min: fold the copy W -> W/2 -> W/4, then reduce
                nc.vector.tensor_tensor(
                    out=s1, in0=cp_t[:, 0:W // 2], in1=cp_t[:, W // 2:W], op=MIN)
                nc.vector.tensor_tensor(
                    out=s2, in0=s1[:, 0:W // 4], in1=s1[:, W // 4:W // 2], op=MIN)
                nc.vector.tensor_reduce(
                    out=rowout[:, b:b + 1], in_=s2, axis=X, op=MIN)

            nc.sync.dma_start(out=rowout_d[:, :], in_=rowout[:, :])
            nc.sync.dma_start(out=colout_d[:, :], in_=colacc[:, :])

    nc.compile()
    _BUILT = nc
    return nc


def _window(c):
    center = c * NPC + NPC // 2
    lo = min(max(0, center - W // 2), N - W)
    return lo, lo + W


def kernel(target, output, cur, substeps):
    from concourse.bass_utils import run_bass_kernel_spmd

    a = np.asarray(target, dtype=np.float32)[0]   # (8192,3) target cloud
    b = np.asarray(output, dtype=np.float32)[0]   # (8192,3) output cloud
    cur = int(np.asarray(cur))
    substeps = int(np.asarray(substeps))

    sa = np.argsort(a[:, 0], kind="stable")
    sb = np.argsort(b[:, 0], kind="stable")
    A = a[sa]                                     # sorted targets
    B = b[sb]                                     # sorted outputs

    bf = ml_dtypes.bfloat16
    w_full = _stationary_rows(A).astype(bf)       # [24, 8192]
    r_full = _moving_rows(B).astype(bf)           # [24, 8192]

    in_maps = []
    for c in range(NCORES):
        lo, hi = _window(c)
        in_maps.append({
            "wts": np.ascontiguousarray(w_full[:, c * NPC:(c + 1) * NPC]),
            "rhs": np.ascontiguousarray(r_full[:, lo:hi]),
        })

    nc = _build()
    results = run_bass_kernel_spmd(nc, in_maps, list(range(NCORES))).results

    A64 = A.astype(np.float64)
    B64 = B.astype(np.float64)
    a2 = np.sum(A64 ** 2, 1)
    b2 = np.sum(B64 ** 2, 1)

    # ---- dist1 (per sorted target) ----
    d1 = np.empty(N, np.float64)
    col_parts = []
    for c in range(NCORES):
        d1[c * NPC:(c + 1) * NPC] = results[c]["rowout"].T.reshape(-1)
        col_parts.append(results[c]["colout"].astype(np.float32).min(axis=0))

    # dist1 certificates: squared x-gap to the uncovered ranks
    bad1 = []
    for c in range(NCORES):
        lo, hi = _window(c)
        t = slice(c * NPC, (c + 1) * NPC)
        gl = (A[t, 0] - B[lo - 1, 0]) ** 2 if lo > 0 else np.full(NPC, np.inf)
        gr = (B[hi, 0] - A[t, 0]) ** 2 if hi < N else np.full(NPC, np.inf)
        fail = d1[t] * CERT_MARGIN > np.minimum(gl, gr)
        bad1.extend((c * NPC + np.nonzero(fail)[0]).tolist())
    for t in bad1:
        d1[t] = np.min(a2[t] + b2 - 2.0 * (B64 @ A64[t]))

    # ---- dist2 (per sorted output) ----
    d2 = np.full(N, np.inf, np.float64)
    cov_lo = np.full(N, N, np.int64)
    cov_hi = np.zeros(N, np.int64)
    for c in range(NCORES):
        lo, hi = _window(c)
        np.minimum.at(d2, np.arange(lo, hi), col_parts[c].astype(np.float64))
        cov_lo[lo:hi] = np.minimum(cov_lo[lo:hi], c * NPC)
        cov_hi[lo:hi] = np.maximum(cov_hi[lo:hi], (c + 1) * NPC)
    gl = np.where(cov_lo > 0, (B[:, 0] - A[np.maximum(cov_lo - 1, 0), 0]) ** 2, np.inf)
    gr = np.where(cov_hi < N, (A[np.minimum(cov_hi, N - 1), 0] - B[:, 0]) ** 2, np.inf)
    bad2 = np.nonzero(d2 * CERT_MARGIN > np.minimum(gl, gr))[0]
    for j in bad2:
        d2[j] = np.min(b2[j] + a2 - 2.0 * (A64 @ B64[j]))

    m1 = np.sqrt(np.maximum(d1, 0.0)).mean()
    m2 = np.sqrt(np.maximum(d2, 0.0)).mean()
    loss = 0.5 * (m1 + m2)
    scale = 10.0 / (0.99 ** (cur // substeps))
    return np.float32(loss * scale)


# revision 22
# speedup vs baseline: 6.0133x; 1.0468x over previous
"""Chamfer loss (nn_ChamferLoss) on 8 Trainium2 NeuronCores.

V3: rank-window pruned brute force.

Host sorts both clouds by x. Core c owns the 1024-target slab of sorted rank
[1024c, 1024c+1024) and scans it against the W=2048 output points nearest in
sorted rank (a window centered on the slab, clipped at the ends). For sorted
gaussian clouds the true nearest neighbour lies inside that window for all but
a handful of points; every point carries a certificate (row-min <= squared
x-gap to the uncovered region) checked on the host, and uncertified points are
recomputed exactly on the host (a few points, exact patch).

Distance tiles are computed on the PE as K=24 bf16 matmuls (3-limb bf16
decomposition of fp32 -> fp32-accurate d2 at full bf16 PE rate).  Per
[128,2048] PSUM tile: ACT evacuates to a bf16 SBUF copy; DVE folds the copy
for the row-min (2x-packed bf16 tensor_tensor min) and accumulates the
column-min across the 8 blocks.  dist1 row-mins and the per-core column-min
window go back to the host, which folds partitions/cores, applies the
certificates, patches, and finishes sqrt/mean/scale.
"""

import sys

sys.path.insert(0, "/opt/trn_rl_repo")

import numpy as np
import ml_dtypes

N = 8192           # points per cloud
NCORES = 8
NPC = N // NCORES  # 1024 targets per core
P = 128
BLKS = NPC // P    # 8 blocks per core
K = 24             # contraction rows (3-limb decomposition)
CH = 512           # matmul free dim (one PSUM bank fp32)
W = 1024           # output-point window per core
CERT_MARGIN = 1.01 # bf16 slack when checking certificates

_BUILT = None


def _limbs(x):
    h = x.astype(ml_dtypes.bfloat16).astype(np.float32)
    r = x - h
    m = r.astype(ml_dtypes.bfloat16).astype(np.float32)
    l = (r - m).astype(ml_dtypes.bfloat16).astype(np.float32)
    return h, m, l


def _stationary_rows(pts):
    """[24, n] lhsT rows: coord limbs + |p|^2 limbs + ones."""
    ph, pm, pl = _limbs(pts)
    p2 = np.sum(pts.astype(np.float64) ** 2, -1).astype(np.float32)
    p2h, p2m, p2l = _limbs(p2)
    one = np.ones_like(p2)
    return np.stack(
        [ph[:, 0], ph[:, 1], ph[:, 2],
         ph[:, 0], ph[:, 1], ph[:, 2],
         pm[:, 0], pm[:, 1], pm[:, 2],
         ph[:, 0], ph[:, 1], ph[:, 2],
         pl[:, 0], pl[:, 1], pl[:, 2],
         pm[:, 0], pm[:, 1], pm[:, 2],
         p2h, p2m, p2l,
         one, one, one], 0)


def _moving_rows(pts):
    """[24, n] rhs rows, limb-paired with _stationary_rows."""
    qh, qm, ql = _limbs(pts)
    q2 = np.sum(pts.astype(np.float64) ** 2, -1).astype(np.float32)
    q2h, q2m, q2l = _limbs(q2)
    one = np.ones_like(q2)
    return np.stack(
        [-2 * qh[:, 0], -2 * qh[:, 1], -2 * qh[:, 2],
         -2 * qm[:, 0], -2 * qm[:, 1], -2 * qm[:, 2],
         -2 * qh[:, 0], -2 * qh[:, 1], -2 * qh[:, 2],
         -2 * ql[:, 0], -2 * ql[:, 1], -2 * ql[:, 2],
         -2 * qh[:, 0], -2 * qh[:, 1], -2 * qh[:, 2],
         -2 * qm[:, 0], -2 * qm[:, 1], -2 * qm[:, 2],
         one, one, one,
         q2h, q2m, q2l], 0)


def _build():
    global _BUILT
    if _BUILT is not None:
        return _BUILT

    import concourse.bacc as bacc
    import concourse.mybir as mybir
    import concourse.tile as tile

    f32 = mybir.dt.float32
    bf16 = mybir.dt.bfloat16
    MIN = mybir.AluOpType.min
    X = mybir.AxisListType.X

    nc = bacc.Bacc(None, target_bir_lowering=False, debug=False)
    wts = nc.declare_dram_parameter("wts", [K, NPC], bf16, isOutput=False)
    rhs = nc.declare_dram_parameter("rhs", [K, W], bf16, isOutput=False)
    rowout_d = nc.declare_dram_parameter("rowout", [P, BLKS * (W // 4)], bf16, isOutput=True)
    colout_d = nc.declare_dram_parameter("colout", [P, W], bf16, isOutput=True)

    with tile.TileContext(nc) as tc:
        with tc.tile_pool(name="const", bufs=1) as cpool, \
             tc.tile_pool(name="cp", bufs=8) as cppool, \
             tc.tile_pool(name="ps", bufs=2, space="PSUM") as pspool:
            w_t = cpool.tile([K, NPC], bf16, name="w_t")
            r_t = cpool.tile([K, W], bf16, name="r_t")
            # parallel queues + chunked rhs so block 0's matmuls start early
            nc.gpsimd.dma_start(out=w_t[:, :], in_=wts[:, :])
            for j in range(W // CH):
                nc.sync.dma_start(
                    out=r_t[:, j * CH:(j + 1) * CH], in_=rhs[:, j * CH:(j + 1) * CH])

            colacc = cpool.tile([P, W], bf16, name="colacc")
            s1 = cpool.tile([P, W // 2], bf16, name="s1")
            rowfold = cpool.tile([P, BLKS * (W // 4)], bf16, name="rowfold")

            for b in range(BLKS):
                lhsT = w_t[:, b * P:(b + 1) * P]
                pst = pspool.tile([P, W], f32, name="pst")
                for j in range(W // CH):
                    nc.tensor.matmul(
                        out=pst[:, j * CH:(j + 1) * CH],
                        lhsT=lhsT,
                        rhs=r_t[:, j * CH:(j + 1) * CH],
                    )
                if b == 0:
                    cp_t = colacc                 # block 0 copy IS the init
                else:
                    cp_t = cppool.tile([P, W], bf16, name="cp", tag="cp")
                nc.scalar.copy(out=cp_t[:, :], in_=pst[:, :])

                # column accumulation first (feeds the final colout DMA)
                if b > 0:
                    nc.vector.tensor_tensor(
                        out=colacc, in0=cp_t, in1=colacc, op=MIN)

                # row-min: fold the copy W -> W/2 -> W/4; host finishes the min
                nc.vector.tensor_tensor(
                    out=s1, in0=cp_t[:, 0:W // 2], in1=cp_t[:, W // 2:W], op=MIN)
                fb = rowfold[:, b * (W // 4):(b + 1) * (W // 4)]
                nc.vector.tensor_tensor(
                    out=fb, in0=s1[:, 0:W // 4], in1=s1[:, W // 4:W // 2], op=MIN)
                nc.sync.dma_start(
                    out=rowout_d[:, b * (W // 4):(b + 1) * (W // 4)], in_=fb)

            nc.sync.dma_start(out=colout_d[:, :], in_=colacc[:, :])

    nc.compile()
    _BUILT = nc
    return nc


def _window(c):
    center = c * NPC + NPC // 2
    lo = min(max(0, center - W // 2), N - W)
    return lo, lo + W


def kernel(target, output, cur, substeps):
    from concourse.bass_utils import run_bass_kernel_spmd

    a = np.asarray(target, dtype=np.float32)[0]   # (8192,3) target cloud
    b = np.asarray(output, dtype=np.float32)[0]   # (8192,3) output cloud
    cur = int(np.asarray(cur))
    substeps = int(np.asarray(substeps))

    sa = np.argsort(a[:, 0], kind="stable")
    sb = np.argsort(b[:, 0], kind="stable")
    A = a[sa]                                     # sorted targets
    B = b[sb]                                     # sorted outputs

    bf = ml_dtypes.bfloat16
    w_full = _stationary_rows(A).astype(bf)       # [24, 8192]
    r_full = _moving_rows(B).astype(bf)           # [24, 8192]

    in_maps = []
    for c in range(NCORES):
        lo, hi = _window(c)
        in_maps.append({
            "wts": np.ascontiguousarray(w_full[:, c * NPC:(c + 1) * NPC]),
            "rhs": np.ascontiguousarray(r_full[:, lo:hi]),
        })

    nc = _build()
    results = run_bass_kernel_spmd(nc, in_maps, list(range(NCORES))).results

    A64 = A.astype(np.float64)
    B64 = B.astype(np.float64)
    a2 = np.sum(A64 ** 2, 1)
    b2 = np.sum(B64 ** 2, 1)

    # ---- dist1 (per sorted target) ----
    d1 = np.empty(N, np.float64)
    col_parts = []
    for c in range(NCORES):
        rf = results[c]["rowout"].astype(np.float32)      # [128, BLKS*(W//4)]
        rmins = rf.reshape(P, BLKS, W // 4).min(axis=2)   # [128, BLKS]
        d1[c * NPC:(c + 1) * NPC] = rmins.T.reshape(-1)
        col_parts.append(results[c]["colout"].astype(np.float32).min(axis=0))

    # dist1 certificates: squared x-gap to the uncovered ranks
    bad1 = []
    for c in range(NCORES):
        lo, hi = _window(c)
        t = slice(c * NPC, (c + 1) * NPC)
        gl = (A[t, 0] - B[lo - 1, 0]) ** 2 if lo > 0 else np.full(NPC, np.inf)
        gr = (B[hi, 0] - A[t, 0]) ** 2 if hi < N else np.full(NPC, np.inf)
        fail = d1[t] * CERT_MARGIN > np.minimum(gl, gr)
        bad1.extend((c * NPC + np.nonzero(fail)[0]).tolist())
    for t in bad1:
        d1[t] = np.min(a2[t] + b2 - 2.0 * (B64 @ A64[t]))

    # ---- dist2 (per sorted output) ----
    d2 = np.full(N, np.inf, np.float64)
    cov_lo = np.full(N, N, np.int64)
    cov_hi = np.zeros(N, np.int64)
    for c in range(NCORES):
        lo, hi = _window(c)
        np.minimum.at(d2, np.arange(lo, hi), col_parts[c].astype(np.float64))
        cov_lo[lo:hi] = np.minimum(cov_lo[lo:hi], c * NPC)
        cov_hi[lo:hi] = np.maximum(cov_hi[lo:hi], (c + 1) * NPC)
    gl = np.where(cov_lo > 0, (B[:, 0] - A[np.maximum(cov_lo - 1, 0), 0]) ** 2, np.inf)
    gr = np.where(cov_hi < N, (A[np.minimum(cov_hi, N - 1), 0] - B[:, 0]) ** 2, np.inf)
    bad2 = np.nonzero(d2 * CERT_MARGIN > np.minimum(gl, gr))[0]
    for j in bad2:
        d2[j] = np.min(b2[j] + a2 - 2.0 * (A64 @ B64[j]))

    m1 = np.sqrt(np.maximum(d1, 0.0)).mean()
    m2 = np.sqrt(np.maximum(d2, 0.0)).mean()
    loss = 0.5 * (m1 + m2)
    scale = 10.0 / (0.99 ** (cur // substeps))
    return np.float32(loss * scale)
